# revision 11
# baseline (speedup 1.0000x reference)
"""Multi-head attention (S=2048, D=1024, H=16, dk=dv=64) on 8 TRN2 NeuronCores.

Sharding: head-parallel tensor parallelism. Core c owns heads {2c, 2c+1}:
  - QT/KT [128, S] (two heads stacked on partitions); V via PE-transpose of
    VT, augmented with a ones column so the ctx matmul also produces the
    softmax denominators (softmax runs over the partition axis).
  - scoresT tiles -> exp on ACT (scale=1/8 folded in) -> ctx accumulation.
  - per s-chunk: normalize ctxT, AllGather the [128, chunk] block across
    cores -> [1024, chunk] concat, then a 128-column slice of the output
    projection per core (outT layout). Host unshard = concat + transpose.

Overlap structure: enc_k/enc_q stream on the two HWDGE queues while enc_v
cast-streams on the SWDGE queue, all from t=0. K and the first half of Q
project first (the minimum needed to start the exp stream); V and the
second Q half project inside the first chunk's scores loop using
time-shared PSUM pools. s is processed in three chunks (1024/512/512) so
the per-chunk ctx AllGathers start early and the last one is small.

Compute dtype: bf16 operands, fp32 PSUM accumulation, softmax in fp32.
"""

import numpy as np

import concourse.bass as bass
import concourse.mybir as mybir
import concourse.tile as tile
from concourse import bacc
from concourse.bass_utils import run_bass_kernel_spmd

S = 2048
D = 1024
H = 16
DK = 64
DV = 64
NCORES = 8
HPC = H // NCORES          # heads per core = 2
FW = HPC * DV              # per-core feature width = 128
P = 128                    # partitions
KT_D = D // P              # 8 contraction tiles over D
TT = S // P                # 16 tiles over t (keys)
NQ = 512                   # matmul moving free dim
CHUNKS = (1024, 512, 512)  # s-chunks (ctx/AG granularity)
VA = 2 * (DV + 1)          # V_aug feature width

F32 = mybir.dt.float32
BF16 = mybir.dt.bfloat16
EXPF = mybir.ActivationFunctionType.Exp

_cache = {}


def build():
    nc = bacc.Bacc(None, target_bir_lowering=False)

    enc_in = {
        x: nc.dram_tensor(f"enc{x}_t", [D, S], F32, kind="ExternalInput")
        for x in ("q", "k", "v")
    }
    w_in = {
        n: nc.dram_tensor(n, [D, FW], F32, kind="ExternalInput")
        for n in ("wq", "wk", "wv", "wo")
    }
    out_t = nc.dram_tensor("outT", [FW, S], F32, kind="ExternalOutput")

    with tile.TileContext(nc) as tc:
        with (
            tc.tile_pool(name="wts", bufs=1) as wts,
            tc.tile_pool(name="encp", bufs=3) as encp,
            tc.tile_pool(name="qkv", bufs=1) as qkv,
            tc.tile_pool(name="expp", bufs=4) as expp,
            tc.tile_pool(name="catp", bufs=1) as catp,
            tc.tile_pool(name="catin", bufs=3) as catin,
            tc.tile_pool(name="misc", bufs=2) as misc,
            tc.tile_pool(name="dram", bufs=1, space="DRAM") as dram,
        ):
            rg = [list(range(NCORES))]

            # ---- weights: cast-DMA f32 -> bf16, [128, KT_D, FW] ----
            wtiles = {}
            for name in ("wq", "wk", "wv", "wo"):
                wt = wts.tile([P, KT_D, FW], BF16, tag=f"w_{name}", name=name)
                nc.gpsimd.dma_start(
                    wt[:], w_in[name].rearrange("(kt p) m -> p kt m", p=P)
                )
                wtiles[name] = wt

            # persistent SBUF state
            qt_sb = qkv.tile([P, S], BF16, tag="qt")
            kt_sb = qkv.tile([P, S], BF16, tag="kt")
            vt_sb = qkv.tile([P, S], BF16, tag="vt")
            v_aug = qkv.tile([P, TT, VA], BF16, tag="vaug")
            cat_loc = catp.tile([P, S], BF16, tag="cat")
            out_sb = catp.tile([P, S], F32, tag="outsb")

            # ---- enc_v: SWDGE cast-DMA stream, queued from t=0 ----
            ev_tiles = []
            for dt in range(KT_D):
                ev = encp.tile([P, S], BF16, tag="encv", bufs=KT_D, name="ev")
                nc.gpsimd.dma_start(ev[:], enc_in["v"][dt * P : (dt + 1) * P, :])
                ev_tiles.append(ev)

            # enc loader: halves on the two HWDGE queues + DVE cast
            def load_enc(x, dt, cols, tagsuf, bufs=3):
                c0, c1 = cols
                w2 = (c1 - c0) // 2
                raw = encp.tile(
                    [P, c1 - c0], F32, tag=f"raw{tagsuf}", bufs=bufs, name="raw"
                )
                nc.sync.dma_start(
                    raw[:, :w2], enc_in[x][dt * P : (dt + 1) * P, c0 : c0 + w2]
                )
                nc.scalar.dma_start(
                    raw[:, w2:], enc_in[x][dt * P : (dt + 1) * P, c0 + w2 : c1]
                )
                t = encp.tile(
                    [P, c1 - c0], BF16, tag=f"bf{tagsuf}", bufs=bufs, name="bf"
                )
                nc.vector.tensor_copy(t[:], raw[:])
                return t

            # ---- phase 0: K full projection, Q first half ----
            ps_p_cm = tc.tile_pool(name="ps_p", bufs=1, space="PSUM")
            ps_p = ps_p_cm.__enter__()
            kacc = {
                sc4: ps_p.tile([P, NQ], F32, tag=f"ka{sc4}", name=f"ka{sc4}")
                for sc4 in range(4)
            }
            for dt in range(KT_D):
                ek = load_enc("k", dt, (0, S), "k")
                for sc4 in range(4):
                    nc.tensor.matmul(
                        kacc[sc4][:],
                        wtiles["wk"][:, dt, :],
                        ek[:, sc4 * NQ : (sc4 + 1) * NQ],
                        start=(dt == 0),
                        stop=(dt == KT_D - 1),
                    )
            for sc4 in range(4):
                nc.vector.tensor_copy(
                    kt_sb[:, sc4 * NQ : (sc4 + 1) * NQ], kacc[sc4][:]
                )
            qacc = {
                nn: ps_p.tile([P, NQ], F32, tag=f"qa{nn}", name=f"qa{nn}")
                for nn in range(2)
            }
            for dt in range(KT_D):
                eq = load_enc("q", dt, (0, 1024), "q")
                for nn in range(2):
                    nc.tensor.matmul(
                        qacc[nn][:],
                        wtiles["wq"][:, dt, :],
                        eq[:, nn * NQ : (nn + 1) * NQ],
                        start=(dt == 0),
                        stop=(dt == KT_D - 1),
                    )
            for nn in range(2):
                nc.vector.tensor_copy(
                    qt_sb[:, nn * NQ : (nn + 1) * NQ], qacc[nn][:]
                )
            ps_p_cm.__exit__(None, None, None)

            # identity + ones columns (first needed by v_finish)
            ident = wts.tile([P, P], BF16, tag="ident")
            from concourse.masks import make_identity

            make_identity(nc, ident)
            nc.any.memset(v_aug[:, :, DV : DV + 1], 1.0)
            nc.any.memset(v_aug[:, :, 2 * DV + 1 : 2 * DV + 2], 1.0)

            # ---- attention with interleaved V / Q-half1 projections ----
            ps_at_cm = tc.tile_pool(name="ps_at", bufs=1, space="PSUM")
            ps_at = ps_at_cm.__enter__()
            ctx_ps = {}

            def scores_tt(ci, tt):
                c0 = sum(CHUNKS[:ci])
                exs = []
                for half in range(CHUNKS[ci] // NQ):
                    m = ps_at.tile([P, 1024], F32, tag="mega", bufs=2, name="m")
                    s0 = c0 + half * NQ
                    for h in range(HPC):
                        nc.tensor.matmul(
                            m[:, h * NQ : (h + 1) * NQ],
                            kt_sb[h * DK : (h + 1) * DK, tt * P : (tt + 1) * P],
                            qt_sb[h * DK : (h + 1) * DK, s0 : s0 + NQ],
                            start=True,
                            stop=True,
                        )
                    ex = expp.tile(
                        [P, 1024], BF16, tag=f"exp{len(exs) % 2}", bufs=4,
                        name="ex",
                    )
                    nc.scalar.activation(
                        ex[:], m[:], EXPF, scale=1.0 / np.sqrt(DK)
                    )
                    exs.append(ex)
                return exs

            def ctx_tt(ci, tt, exs):
                for h in range(HPC):
                    for half, ex in enumerate(exs):
                        nc.tensor.matmul(
                            ctx_ps[(ci, h)][:, half * NQ : (half + 1) * NQ],
                            v_aug[:, tt, h * (DV + 1) : (h + 1) * (DV + 1)],
                            ex[:, h * NQ : (h + 1) * NQ],
                            start=(tt == 0),
                            stop=(tt == TT - 1),
                        )

            def normalize(ci):
                cw = CHUNKS[ci]
                c0 = sum(CHUNKS[:ci])
                for h in range(HPC):
                    den = misc.tile([1, cw], F32, tag="den", name="den")
                    nc.vector.tensor_copy(
                        den[:], ctx_ps[(ci, h)][DV : DV + 1, 0:cw]
                    )
                    recip = misc.tile([1, cw], F32, tag="recip", name="recip")
                    nc.vector.reciprocal_approx_fast(recip[:], den[:])
                    bcast = misc.tile([DV, cw], F32, tag="bcast", name="bcast")
                    nc.gpsimd.partition_broadcast(bcast[:], recip[:])
                    nc.vector.tensor_mul(
                        cat_loc[h * DV : (h + 1) * DV, c0 : c0 + cw],
                        ctx_ps[(ci, h)][0:DV, 0:cw],
                        bcast[:],
                    )
                cb = dram.tile([P, cw], BF16, tag=f"catb{ci}", name="cb")
                nc.sync.dma_start(cb[:], cat_loc[:, c0 : c0 + cw])
                ga = dram.tile([D, cw], BF16, tag=f"catall{ci}", name="ga")
                nc.gpsimd.collective_compute(
                    "AllGather",
                    mybir.AluOpType.bypass,
                    ins=[cb[:].opt()],
                    outs=[ga[:].opt()],
                    replica_groups=rg,
                )
                return ga

            def outproj(ci, ga):
                cw = CHUNKS[ci]
                c0 = sum(CHUNKS[:ci])
                m = ps_at.tile([P, 1024], F32, tag="mega", bufs=2, name="om")
                for kt in range(KT_D):
                    ct = catin.tile([P, cw], BF16, tag="catkt", name="ct")
                    nc.sync.dma_start(ct[:], ga[kt * P : (kt + 1) * P, :])
                    for nn in range(cw // NQ):
                        nc.tensor.matmul(
                            m[:, nn * NQ : (nn + 1) * NQ],
                            wtiles["wo"][:, kt, :],
                            ct[:, nn * NQ : (nn + 1) * NQ],
                            start=(kt == 0),
                            stop=(kt == KT_D - 1),
                        )
                nc.vector.tensor_copy(out_sb[:, c0 : c0 + cw], m[:, 0:cw])
                nc.sync.dma_start(
                    out_t[:, c0 : c0 + cw], out_sb[:, c0 : c0 + cw]
                )

            # --- chunk 0 scores, V then Q-half1 interleaved ---
            ps_v2_cm = tc.tile_pool(name="ps_v2", bufs=1, space="PSUM")
            ps_v2 = ps_v2_cm.__enter__()
            vacc = {
                half: ps_v2.tile(
                    [P, 1024], F32, tag=f"va{half}", name=f"va{half}"
                )
                for half in range(2)
            }

            def v_partial(dt):
                ev = ev_tiles[dt]
                for half in range(2):
                    for nn in range(2):
                        off = half * 1024 + nn * NQ
                        nc.tensor.matmul(
                            vacc[half][:, nn * NQ : (nn + 1) * NQ],
                            wtiles["wv"][:, dt, :],
                            ev[:, off : off + NQ],
                            start=(dt == 0),
                            stop=(dt == KT_D - 1),
                        )

            def v_finish():
                for half in range(2):
                    for nn in range(2):
                        off = half * 1024 + nn * NQ
                        nc.vector.tensor_copy(
                            vt_sb[:, off : off + NQ],
                            vacc[half][:, nn * NQ : (nn + 1) * NQ],
                        )
                for tt in range(TT):
                    tp = ps_v2.tile([P, P], BF16, tag="va0", name="tp")
                    nc.tensor.transpose(
                        tp[:], vt_sb[:, tt * P : (tt + 1) * P], ident[:]
                    )
                    nc.vector.tensor_copy(v_aug[:, tt, 0:DV], tp[:, 0:DV])
                    nc.vector.tensor_copy(
                        v_aug[:, tt, DV + 1 : 2 * DV + 1], tp[:, DV : 2 * DV]
                    )

            sc0_exs = []
            ps_q2_cm = None
            for tt in range(TT):
                sc0_exs.append(scores_tt(0, tt))
                if tt < 8:
                    v_partial(tt)
                elif tt == 8:
                    v_finish()
                    ps_v2_cm.__exit__(None, None, None)
                    ps_q2_cm = tc.tile_pool(name="ps_q2", bufs=1, space="PSUM")
                    ps_q2 = ps_q2_cm.__enter__()
                    q2acc = ps_q2.tile([P, 1024], F32, tag="q2a", name="q2a")
                elif tt < 15:
                    dt = tt - 9  # 0..5
                    eq = load_enc("q", dt, (1024, 2048), "q")
                    for nn in range(2):
                        nc.tensor.matmul(
                            q2acc[:, nn * NQ : (nn + 1) * NQ],
                            wtiles["wq"][:, dt, :],
                            eq[:, nn * NQ : (nn + 1) * NQ],
                            start=(dt == 0),
                            stop=False,
                        )
            for dt in (6, 7):
                eq = load_enc("q", dt, (1024, 2048), "q")
                for nn in range(2):
                    nc.tensor.matmul(
                        q2acc[:, nn * NQ : (nn + 1) * NQ],
                        wtiles["wq"][:, dt, :],
                        eq[:, nn * NQ : (nn + 1) * NQ],
                        start=False,
                        stop=(dt == 7),
                    )
            for nn in range(2):
                nc.vector.tensor_copy(
                    qt_sb[:, 1024 + nn * NQ : 1024 + (nn + 1) * NQ],
                    q2acc[:, nn * NQ : (nn + 1) * NQ],
                )
            ps_q2_cm.__exit__(None, None, None)

            # ctx pool (time-shares the banks V/Q-half1 used)
            ps_cx_cm = tc.tile_pool(name="ps_cx", bufs=1, space="PSUM")
            ps_cx = ps_cx_cm.__enter__()

            def alloc_ctx(ci):
                for h in range(HPC):
                    ctx_ps[(ci, h)] = ps_cx.tile(
                        [DV + 1, 1024], F32, tag=f"cx{h}", name=f"cx{h}"
                    )

            # --- chunk 1 scores + ctx(0) at double rate ---
            alloc_ctx(0)
            sc1_exs = []
            for tt in range(TT):
                sc1_exs.append(scores_tt(1, tt))
                for k in (2 * tt, 2 * tt + 1):
                    if k < TT:
                        ctx_tt(0, k, sc0_exs[k])
            ga0 = normalize(0)

            # --- chunk 2 scores + ctx(1) ---
            alloc_ctx(1)
            sc2_exs = []
            for tt in range(TT):
                sc2_exs.append(scores_tt(2, tt))
                ctx_tt(1, tt, sc1_exs[tt])
            ga1 = normalize(1)
            alloc_ctx(2)
            for tt in range(TT):
                ctx_tt(2, tt, sc2_exs[tt])
            ga2 = normalize(2)
            outproj(0, ga0)
            outproj(1, ga1)
            outproj(2, ga2)
            ps_cx_cm.__exit__(None, None, None)
            ps_at_cm.__exit__(None, None, None)

    nc.compile()
    return nc


def kernel(
    encodings_for_q,
    encodings_for_k,
    encodings_for_v,
    W_q,
    W_k,
    W_v,
    W_out,
    _trace: bool = False,
):
    encodings_for_q = np.asarray(encodings_for_q, dtype=np.float32)
    encodings_for_k = np.asarray(encodings_for_k, dtype=np.float32)
    encodings_for_v = np.asarray(encodings_for_v, dtype=np.float32)
    W_q = np.asarray(W_q, dtype=np.float32)
    W_k = np.asarray(W_k, dtype=np.float32)
    W_v = np.asarray(W_v, dtype=np.float32)
    W_out = np.asarray(W_out, dtype=np.float32)

    if "nc" not in _cache:
        _cache["nc"] = build()
    nc = _cache["nc"]

    eqT = np.ascontiguousarray(encodings_for_q.T)
    ekT = np.ascontiguousarray(encodings_for_k.T)
    evT = np.ascontiguousarray(encodings_for_v.T)

    in_maps = []
    for c in range(NCORES):
        hs = slice(HPC * c, HPC * (c + 1))
        in_maps.append(
            {
                "encq_t": eqT,
                "enck_t": ekT,
                "encv_t": evT,
                "wq": np.ascontiguousarray(
                    np.transpose(W_q[hs], (1, 0, 2)).reshape(D, FW)
                ),
                "wk": np.ascontiguousarray(
                    np.transpose(W_k[hs], (1, 0, 2)).reshape(D, FW)
                ),
                "wv": np.ascontiguousarray(
                    np.transpose(W_v[hs], (1, 0, 2)).reshape(D, FW)
                ),
                "wo": np.ascontiguousarray(W_out[:, FW * c : FW * (c + 1)]),
            }
        )

    r = run_bass_kernel_spmd(
        nc, in_maps, core_ids=list(range(NCORES)), trace=_trace
    )
    out = np.concatenate(
        [r.results[c]["outT"].T for c in range(NCORES)], axis=1
    )
    if _trace:
        kernel.last_exec_time_ns = r.exec_time_ns
        kernel.last_insts = (
            r.instructions_and_trace[0] if r.instructions_and_trace else None
        )
    return out.astype(np.float32)


# revision 13
# speedup vs baseline: 1.2344x; 1.2344x over previous
"""Multi-head attention (S=2048, D=1024, H=16, dk=dv=64) on 8 TRN2 NeuronCores.

Sharding: head-parallel tensor parallelism. Core c owns heads {2c, 2c+1}:
  - QT/KT [128, S] (two heads stacked on partitions); V via PE-transpose of
    VT, augmented with a ones column so the ctx matmul also produces the
    softmax denominators (softmax runs over the partition axis).
  - scoresT tiles -> exp on ACT (scale=1/8 folded in) -> ctx accumulation.
  - per s-chunk: normalize ctxT, AllGather the [128, chunk] block across
    cores -> [1024, chunk] concat, then a 128-column slice of the output
    projection per core (outT layout). Host unshard = concat + transpose.

Overlap structure: enc_k/enc_q stream on the two HWDGE queues while enc_v
cast-streams on the SWDGE queue, all from t=0. K and the first half of Q
project first (the minimum needed to start the exp stream); V and the
second Q half project inside the first chunk's scores loop using
time-shared PSUM pools. s is processed in three chunks (1024/512/512) so
the per-chunk ctx AllGathers start early and the last one is small.

Compute dtype: bf16 operands, fp32 PSUM accumulation, softmax in fp32.
"""

import numpy as np

import concourse.bass as bass
import concourse.mybir as mybir
import concourse.tile as tile
from concourse import bacc
from concourse.bass_utils import run_bass_kernel_spmd

S = 2048
D = 1024
H = 16
DK = 64
DV = 64
NCORES = 8
HPC = H // NCORES          # heads per core = 2
FW = HPC * DV              # per-core feature width = 128
P = 128                    # partitions
KT_D = D // P              # 8 contraction tiles over D
TT = S // P                # 16 tiles over t (keys)
NQ = 512                   # matmul moving free dim
CHUNKS = (1024, 512, 512)  # s-chunks (ctx/AG granularity)
VA = 2 * (DV + 1)          # V_aug feature width

F32 = mybir.dt.float32
BF16 = mybir.dt.bfloat16
EXPF = mybir.ActivationFunctionType.Exp

_cache = {}


def build():
    nc = bacc.Bacc(None, target_bir_lowering=False)

    enc_in = {
        x: nc.dram_tensor(f"enc{x}_t", [D, S], F32, kind="ExternalInput")
        for x in ("q", "k", "v")
    }
    w_in = {
        n: nc.dram_tensor(n, [D, FW], F32, kind="ExternalInput")
        for n in ("wq", "wk", "wv", "wo")
    }
    out_t = nc.dram_tensor("outT", [FW, S], F32, kind="ExternalOutput")

    with tile.TileContext(nc) as tc:
        with (
            tc.tile_pool(name="wts", bufs=1) as wts,
            tc.tile_pool(name="encp", bufs=3) as encp,
            tc.tile_pool(name="qkv", bufs=1) as qkv,
            tc.tile_pool(name="expp", bufs=16) as expp,
            tc.tile_pool(name="catp", bufs=1) as catp,
            tc.tile_pool(name="catin", bufs=3) as catin,
            tc.tile_pool(name="misc", bufs=1) as misc,
            tc.tile_pool(name="dram", bufs=1, space="DRAM") as dram,
        ):
            rg = [list(range(NCORES))]

            # ---- weights: cast-DMA f32 -> bf16, [128, KT_D, FW] ----
            wtiles = {}
            for name in ("wq", "wk", "wv", "wo"):
                wt = wts.tile([P, KT_D, FW], BF16, tag=f"w_{name}", name=name)
                nc.gpsimd.dma_start(
                    wt[:], w_in[name].rearrange("(kt p) m -> p kt m", p=P)
                )
                wtiles[name] = wt

            # persistent SBUF state
            qt_sb = qkv.tile([P, S], BF16, tag="qt")
            kt_sb = qkv.tile([P, S], BF16, tag="kt")
            vt_sb = qkv.tile([P, S], BF16, tag="vt")
            v_aug = qkv.tile([P, TT, VA], BF16, tag="vaug")
            cat_loc = catp.tile([P, S], BF16, tag="cat")
            out_sb = catp.tile([P, S], F32, tag="outsb")

            # enc loader: halves on the two HWDGE queues + DVE cast
            def load_enc(x, dt, cols, tagsuf, bufs=3):
                c0, c1 = cols
                w2 = (c1 - c0) // 2
                raw = encp.tile(
                    [P, c1 - c0], F32, tag=f"raw{tagsuf}", bufs=bufs, name="raw"
                )
                nc.sync.dma_start(
                    raw[:, :w2], enc_in[x][dt * P : (dt + 1) * P, c0 : c0 + w2]
                )
                nc.scalar.dma_start(
                    raw[:, w2:], enc_in[x][dt * P : (dt + 1) * P, c0 + w2 : c1]
                )
                t = encp.tile(
                    [P, c1 - c0], BF16, tag=f"bf{tagsuf}", bufs=bufs, name="bf"
                )
                nc.vector.tensor_copy(t[:], raw[:])
                return t

            # ---- phase 0: K full projection, Q first half ----
            ps_p_cm = tc.tile_pool(name="ps_p", bufs=1, space="PSUM")
            ps_p = ps_p_cm.__enter__()
            kacc = {
                sc4: ps_p.tile([P, NQ], F32, tag=f"ka{sc4}", name=f"ka{sc4}")
                for sc4 in range(4)
            }
            # PE warm-up: ~5us of junk matmuls on the first K tile so the
            # HAM un-throttles before the real stream begins (LDW-broken
            # streams never trigger the busy window; once warm, short gaps
            # don't re-throttle).
            ek0 = load_enc("k", 0, (0, S), "k")
            wm = ps_p.tile([P, NQ], F32, tag="warm", name="wm")
            for i in range(24):
                nc.tensor.matmul(
                    wm[:], wtiles["wk"][:, 0, :], ek0[:, 0:NQ],
                    start=True, stop=True,
                )
            for dt in range(KT_D):
                ek = ek0 if dt == 0 else load_enc("k", dt, (0, S), "k")
                for sc4 in range(4):
                    nc.tensor.matmul(
                        kacc[sc4][:],
                        wtiles["wk"][:, dt, :],
                        ek[:, sc4 * NQ : (sc4 + 1) * NQ],
                        start=(dt == 0),
                        stop=(dt == KT_D - 1),
                    )
            for sc4 in range(4):
                nc.vector.tensor_copy(
                    kt_sb[:, sc4 * NQ : (sc4 + 1) * NQ], kacc[sc4][:]
                )
            qacc = {
                nn: ps_p.tile([P, NQ], F32, tag=f"qa{nn}", name=f"qa{nn}")
                for nn in range(2)
            }
            for dt in range(KT_D):
                eq = load_enc("q", dt, (0, 1024), "q")
                for nn in range(2):
                    nc.tensor.matmul(
                        qacc[nn][:],
                        wtiles["wq"][:, dt, :],
                        eq[:, nn * NQ : (nn + 1) * NQ],
                        start=(dt == 0),
                        stop=(dt == KT_D - 1),
                    )
            for nn in range(2):
                nc.vector.tensor_copy(
                    qt_sb[:, nn * NQ : (nn + 1) * NQ], qacc[nn][:]
                )
            ps_p_cm.__exit__(None, None, None)

            # identity + ones columns (first needed by v_finish)
            ident = wts.tile([P, P], BF16, tag="ident")
            from concourse.masks import make_identity

            make_identity(nc, ident)
            nc.any.memset(v_aug[:, :, DV : DV + 1], 1.0)
            nc.any.memset(v_aug[:, :, 2 * DV + 1 : 2 * DV + 2], 1.0)

            # ---- attention with interleaved V / Q-half1 projections ----
            ps_at_cm = tc.tile_pool(name="ps_at", bufs=1, space="PSUM")
            ps_at = ps_at_cm.__enter__()
            ctx_ps = {}

            def scores_tt(ci, tt):
                c0 = sum(CHUNKS[:ci])
                exs = []
                for half in range(CHUNKS[ci] // NQ):
                    m = ps_at.tile([P, 1024], F32, tag="mega", bufs=2, name="m")
                    s0 = c0 + half * NQ
                    for h in range(HPC):
                        nc.tensor.matmul(
                            m[:, h * NQ : (h + 1) * NQ],
                            kt_sb[h * DK : (h + 1) * DK, tt * P : (tt + 1) * P],
                            qt_sb[h * DK : (h + 1) * DK, s0 : s0 + NQ],
                            start=True,
                            stop=True,
                        )
                    ex = expp.tile(
                        [P, 1024], BF16, tag=f"exp{len(exs) % 2}", bufs=16,
                        name="ex",
                    )
                    nc.scalar.activation(
                        ex[:], m[:], EXPF, scale=1.0 / np.sqrt(DK)
                    )
                    exs.append(ex)
                return exs

            def ctx_tt(ci, tt, exs):
                for h in range(HPC):
                    for half, ex in enumerate(exs):
                        nc.tensor.matmul(
                            ctx_ps[(ci, h)][:, half * NQ : (half + 1) * NQ],
                            v_aug[:, tt, h * (DV + 1) : (h + 1) * (DV + 1)],
                            ex[:, h * NQ : (h + 1) * NQ],
                            start=(tt == 0),
                            stop=(tt == TT - 1),
                        )

            def normalize(ci):
                cw = CHUNKS[ci]
                c0 = sum(CHUNKS[:ci])
                for h in range(HPC):
                    den = misc.tile([1, cw], F32, tag="den", name="den")
                    nc.vector.tensor_copy(
                        den[:], ctx_ps[(ci, h)][DV : DV + 1, 0:cw]
                    )
                    recip = misc.tile([1, cw], F32, tag="recip", name="recip")
                    nc.vector.reciprocal_approx_fast(recip[:], den[:])
                    bcast = misc.tile([DV, cw], F32, tag="bcast", name="bcast")
                    nc.gpsimd.partition_broadcast(bcast[:], recip[:])
                    nc.vector.tensor_mul(
                        cat_loc[h * DV : (h + 1) * DV, c0 : c0 + cw],
                        ctx_ps[(ci, h)][0:DV, 0:cw],
                        bcast[:],
                    )
                cb = dram.tile([P, cw], BF16, tag=f"catb{ci}", name="cb")
                nc.sync.dma_start(cb[:], cat_loc[:, c0 : c0 + cw])
                ga = dram.tile([D, cw], BF16, tag=f"catall{ci}", name="ga")
                nc.gpsimd.collective_compute(
                    "AllGather",
                    mybir.AluOpType.bypass,
                    ins=[cb[:].opt()],
                    outs=[ga[:].opt()],
                    replica_groups=rg,
                )
                return ga

            def outproj(ci, ga):
                cw = CHUNKS[ci]
                c0 = sum(CHUNKS[:ci])
                m = ps_at.tile([P, 1024], F32, tag="mega", bufs=2, name="om")
                for kt in range(KT_D):
                    ct = catin.tile([P, cw], BF16, tag="catkt", name="ct")
                    nc.sync.dma_start(ct[:], ga[kt * P : (kt + 1) * P, :])
                    for nn in range(cw // NQ):
                        nc.tensor.matmul(
                            m[:, nn * NQ : (nn + 1) * NQ],
                            wtiles["wo"][:, kt, :],
                            ct[:, nn * NQ : (nn + 1) * NQ],
                            start=(kt == 0),
                            stop=(kt == KT_D - 1),
                        )
                nc.vector.tensor_copy(out_sb[:, c0 : c0 + cw], m[:, 0:cw])
                nc.sync.dma_start(
                    out_t[:, c0 : c0 + cw], out_sb[:, c0 : c0 + cw]
                )

            # --- chunk 0 scores, V then Q-half1 interleaved ---
            ps_v2_cm = tc.tile_pool(name="ps_v2", bufs=1, space="PSUM")
            ps_v2 = ps_v2_cm.__enter__()
            vacc = {
                half: ps_v2.tile(
                    [P, 1024], F32, tag=f"va{half}", name=f"va{half}"
                )
                for half in range(2)
            }

            def v_partial(dt):
                ev = load_enc("v", dt, (0, S), "v", bufs=2)
                for half in range(2):
                    for nn in range(2):
                        off = half * 1024 + nn * NQ
                        nc.tensor.matmul(
                            vacc[half][:, nn * NQ : (nn + 1) * NQ],
                            wtiles["wv"][:, dt, :],
                            ev[:, off : off + NQ],
                            start=(dt == 0),
                            stop=(dt == KT_D - 1),
                        )

            def v_finish():
                for half in range(2):
                    for nn in range(2):
                        off = half * 1024 + nn * NQ
                        nc.vector.tensor_copy(
                            vt_sb[:, off : off + NQ],
                            vacc[half][:, nn * NQ : (nn + 1) * NQ],
                        )
                for tt in range(TT):
                    tp = ps_v2.tile([P, P], BF16, tag="va0", name="tp")
                    nc.tensor.transpose(
                        tp[:], vt_sb[:, tt * P : (tt + 1) * P], ident[:]
                    )
                    nc.vector.tensor_copy(v_aug[:, tt, 0:DV], tp[:, 0:DV])
                    nc.vector.tensor_copy(
                        v_aug[:, tt, DV + 1 : 2 * DV + 1], tp[:, DV : 2 * DV]
                    )

            sc0_exs = []
            ps_q2_cm = None
            for tt in range(TT):
                sc0_exs.append(scores_tt(0, tt))
                if tt < 8:
                    v_partial(tt)
                elif tt == 8:
                    v_finish()
                    ps_v2_cm.__exit__(None, None, None)
                    ps_q2_cm = tc.tile_pool(name="ps_q2", bufs=1, space="PSUM")
                    ps_q2 = ps_q2_cm.__enter__()
                    q2acc = ps_q2.tile([P, 1024], F32, tag="q2a", name="q2a")
                elif tt < 15:
                    dt = tt - 9  # 0..5
                    eq = load_enc("q", dt, (1024, 2048), "q")
                    for nn in range(2):
                        nc.tensor.matmul(
                            q2acc[:, nn * NQ : (nn + 1) * NQ],
                            wtiles["wq"][:, dt, :],
                            eq[:, nn * NQ : (nn + 1) * NQ],
                            start=(dt == 0),
                            stop=False,
                        )
            for dt in (6, 7):
                eq = load_enc("q", dt, (1024, 2048), "q")
                for nn in range(2):
                    nc.tensor.matmul(
                        q2acc[:, nn * NQ : (nn + 1) * NQ],
                        wtiles["wq"][:, dt, :],
                        eq[:, nn * NQ : (nn + 1) * NQ],
                        start=False,
                        stop=(dt == 7),
                    )
            for nn in range(2):
                nc.vector.tensor_copy(
                    qt_sb[:, 1024 + nn * NQ : 1024 + (nn + 1) * NQ],
                    q2acc[:, nn * NQ : (nn + 1) * NQ],
                )
            ps_q2_cm.__exit__(None, None, None)

            # ctx pool (time-shares the banks V/Q-half1 used)
            ps_cx_cm = tc.tile_pool(name="ps_cx", bufs=1, space="PSUM")
            ps_cx = ps_cx_cm.__enter__()

            def alloc_ctx(ci):
                for h in range(HPC):
                    ctx_ps[(ci, h)] = ps_cx.tile(
                        [DV + 1, 1024], F32, tag=f"cx{h}", name=f"cx{h}"
                    )

            # --- chunk 1 scores + ctx(0) at double rate ---
            alloc_ctx(0)
            sc1_exs = []
            for tt in range(TT):
                sc1_exs.append(scores_tt(1, tt))
                for k in (2 * tt, 2 * tt + 1):
                    if k < TT:
                        ctx_tt(0, k, sc0_exs[k])
            ga0 = normalize(0)

            # --- chunk 2 scores + ctx(1) ---
            alloc_ctx(1)
            sc2_exs = []
            for tt in range(TT):
                sc2_exs.append(scores_tt(2, tt))
                ctx_tt(1, tt, sc1_exs[tt])
            ga1 = normalize(1)
            alloc_ctx(2)
            for tt in range(TT):
                ctx_tt(2, tt, sc2_exs[tt])
            ga2 = normalize(2)
            outproj(0, ga0)
            outproj(1, ga1)
            outproj(2, ga2)
            ps_cx_cm.__exit__(None, None, None)
            ps_at_cm.__exit__(None, None, None)

    nc.compile()
    return nc


def kernel(
    encodings_for_q,
    encodings_for_k,
    encodings_for_v,
    W_q,
    W_k,
    W_v,
    W_out,
    _trace: bool = False,
):
    encodings_for_q = np.asarray(encodings_for_q, dtype=np.float32)
    encodings_for_k = np.asarray(encodings_for_k, dtype=np.float32)
    encodings_for_v = np.asarray(encodings_for_v, dtype=np.float32)
    W_q = np.asarray(W_q, dtype=np.float32)
    W_k = np.asarray(W_k, dtype=np.float32)
    W_v = np.asarray(W_v, dtype=np.float32)
    W_out = np.asarray(W_out, dtype=np.float32)

    if "nc" not in _cache:
        _cache["nc"] = build()
    nc = _cache["nc"]

    eqT = np.ascontiguousarray(encodings_for_q.T)
    ekT = np.ascontiguousarray(encodings_for_k.T)
    evT = np.ascontiguousarray(encodings_for_v.T)

    in_maps = []
    for c in range(NCORES):
        hs = slice(HPC * c, HPC * (c + 1))
        in_maps.append(
            {
                "encq_t": eqT,
                "enck_t": ekT,
                "encv_t": evT,
                "wq": np.ascontiguousarray(
                    np.transpose(W_q[hs], (1, 0, 2)).reshape(D, FW)
                ),
                "wk": np.ascontiguousarray(
                    np.transpose(W_k[hs], (1, 0, 2)).reshape(D, FW)
                ),
                "wv": np.ascontiguousarray(
                    np.transpose(W_v[hs], (1, 0, 2)).reshape(D, FW)
                ),
                "wo": np.ascontiguousarray(W_out[:, FW * c : FW * (c + 1)]),
            }
        )

    r = run_bass_kernel_spmd(
        nc, in_maps, core_ids=list(range(NCORES)), trace=_trace
    )
    out = np.concatenate(
        [r.results[c]["outT"].T for c in range(NCORES)], axis=1
    )
    if _trace:
        kernel.last_exec_time_ns = r.exec_time_ns
        kernel.last_insts = (
            r.instructions_and_trace[0] if r.instructions_and_trace else None
        )
    return out.astype(np.float32)


# revision 15
# speedup vs baseline: 1.4608x; 1.1833x over previous
"""Multi-head attention (S=2048, D=1024, H=16, dk=dv=64) on 8 TRN2 NeuronCores.

Sharding: head-parallel tensor parallelism. Core c owns heads {2c, 2c+1}:
  - QT/KT [128, S] (two heads stacked on partitions); V via PE-transpose of
    VT, augmented with a ones column so the ctx matmul also produces the
    softmax denominators (softmax runs over the partition axis).
  - scoresT tiles -> exp on ACT (scale=1/8 folded in) -> ctx accumulation.
  - per s-chunk: normalize ctxT, AllGather the [128, chunk] block across
    cores -> [1024, chunk] concat, then a 128-column slice of the output
    projection per core (outT layout). Host unshard = concat + transpose.

Overlap structure: enc_k/enc_q stream on the two HWDGE queues while enc_v
cast-streams on the SWDGE queue, all from t=0. K and the first half of Q
project first (the minimum needed to start the exp stream); V and the
second Q half project inside the first chunk's scores loop using
time-shared PSUM pools. s is processed in three chunks (1024/512/512) so
the per-chunk ctx AllGathers start early and the last one is small.

Compute dtype: bf16 operands, fp32 PSUM accumulation, softmax in fp32.
"""

import numpy as np

import concourse.bass as bass
import concourse.mybir as mybir
import concourse.tile as tile
from concourse import bacc
from concourse.bass_utils import run_bass_kernel_spmd

S = 2048
D = 1024
H = 16
DK = 64
DV = 64
NCORES = 8
HPC = H // NCORES          # heads per core = 2
FW = HPC * DV              # per-core feature width = 128
P = 128                    # partitions
KT_D = D // P              # 8 contraction tiles over D
TT = S // P                # 16 tiles over t (keys)
NQ = 512                   # matmul moving free dim
CHUNKS = (1024, 512, 512)  # s-chunks (ctx/AG granularity)
VA = 2 * (DV + 1)          # V_aug feature width

F32 = mybir.dt.float32
BF16 = mybir.dt.bfloat16
EXPF = mybir.ActivationFunctionType.Exp

_cache = {}


def _prep_w(w):
    """[D, FW] -> [128, KT_D*FW]: row p holds all d-tiles' row p."""
    return np.ascontiguousarray(
        np.transpose(w.reshape(KT_D, P, FW), (1, 0, 2)).reshape(P, KT_D * FW)
    )


def build():
    nc = bacc.Bacc(None, target_bir_lowering=False)

    enc_in = {
        x: nc.dram_tensor(f"enc{x}_t", [D, S], F32, kind="ExternalInput")
        for x in ("q", "k", "v")
    }
    # host pre-arranges weights to [128, KT_D * FW] (d-tile-major columns)
    w_in = {
        n: nc.dram_tensor(n, [P, KT_D * FW], F32, kind="ExternalInput")
        for n in ("wq", "wk", "wv", "wo")
    }
    out_t = nc.dram_tensor("outT", [FW, S], F32, kind="ExternalOutput")

    with tile.TileContext(nc) as tc:
        with (
            tc.tile_pool(name="wts", bufs=1) as wts,
            tc.tile_pool(name="encp", bufs=3) as encp,
            tc.tile_pool(name="qkv", bufs=1) as qkv,
            tc.tile_pool(name="expp", bufs=16) as expp,
            tc.tile_pool(name="catp", bufs=1) as catp,
            tc.tile_pool(name="catin", bufs=3) as catin,
            tc.tile_pool(name="misc", bufs=1) as misc,
            tc.tile_pool(name="dram", bufs=1, space="DRAM") as dram,
        ):
            rg = [list(range(NCORES))]

            # ---- weights: contiguous f32 DMA + DVE cast -> bf16 ----
            wtiles = {}
            for name in ("wq", "wk", "wv", "wo"):
                wraw = encp.tile([P, KT_D * FW], F32, tag="rawq", bufs=3, name=name)
                nc.sync.dma_start(wraw[:], w_in[name][:])
                wt = wts.tile([P, KT_D, FW], BF16, tag=f"w_{name}", name=name)
                nc.vector.tensor_copy(
                    wt[:], wraw.rearrange("p (kt m) -> p kt m", kt=KT_D)
                )
                wtiles[name] = wt

            # persistent SBUF state
            qt_sb = qkv.tile([P, S], BF16, tag="qt")
            kt_sb = qkv.tile([P, S], BF16, tag="kt")
            vt_sb = qkv.tile([P, S], BF16, tag="vt")
            v_aug = qkv.tile([P, TT, VA], BF16, tag="vaug")
            cat_loc = catp.tile([P, S], BF16, tag="cat")
            out_sb = catp.tile([P, S], F32, tag="outsb")

            # enc loader: halves on the two HWDGE queues + DVE cast
            def load_enc(x, dt, cols, tagsuf, bufs=3):
                c0, c1 = cols
                w2 = (c1 - c0) // 2
                raw = encp.tile(
                    [P, c1 - c0], F32, tag=f"raw{tagsuf}", bufs=bufs, name="raw"
                )
                nc.sync.dma_start(
                    raw[:, :w2], enc_in[x][dt * P : (dt + 1) * P, c0 : c0 + w2]
                )
                nc.scalar.dma_start(
                    raw[:, w2:], enc_in[x][dt * P : (dt + 1) * P, c0 + w2 : c1]
                )
                t = encp.tile(
                    [P, c1 - c0], BF16, tag=f"bf{tagsuf}", bufs=bufs, name="bf"
                )
                nc.vector.tensor_copy(t[:], raw[:])
                return t

            # ---- phase 0: K full projection, Q first half ----
            ps_p_cm = tc.tile_pool(name="ps_p", bufs=1, space="PSUM")
            ps_p = ps_p_cm.__enter__()
            kacc = {
                sc4: ps_p.tile([P, NQ], F32, tag=f"ka{sc4}", name=f"ka{sc4}")
                for sc4 in range(4)
            }
            # PE warm-up: ~5us of junk matmuls on the first K tile so the
            # HAM un-throttles before the real stream begins (LDW-broken
            # streams never trigger the busy window; once warm, short gaps
            # don't re-throttle).
            ek0 = load_enc("k", 0, (0, S), "k")
            wm = ps_p.tile([P, NQ], F32, tag="warm", name="wm")
            for i in range(24):
                nc.tensor.matmul(
                    wm[:], wtiles["wk"][:, 0, :], ek0[:, 0:NQ],
                    start=True, stop=True,
                )
            for dt in range(KT_D):
                ek = ek0 if dt == 0 else load_enc("k", dt, (0, S), "k")
                for sc4 in range(4):
                    nc.tensor.matmul(
                        kacc[sc4][:],
                        wtiles["wk"][:, dt, :],
                        ek[:, sc4 * NQ : (sc4 + 1) * NQ],
                        start=(dt == 0),
                        stop=(dt == KT_D - 1),
                    )
            for sc4 in range(4):
                nc.vector.tensor_copy(
                    kt_sb[:, sc4 * NQ : (sc4 + 1) * NQ], kacc[sc4][:]
                )
            qacc = {
                nn: ps_p.tile([P, NQ], F32, tag=f"qa{nn}", name=f"qa{nn}")
                for nn in range(2)
            }
            for dt in range(KT_D):
                eq = load_enc("q", dt, (0, 1024), "q")
                for nn in range(2):
                    nc.tensor.matmul(
                        qacc[nn][:],
                        wtiles["wq"][:, dt, :],
                        eq[:, nn * NQ : (nn + 1) * NQ],
                        start=(dt == 0),
                        stop=(dt == KT_D - 1),
                    )
            for nn in range(2):
                nc.vector.tensor_copy(
                    qt_sb[:, nn * NQ : (nn + 1) * NQ], qacc[nn][:]
                )
            ps_p_cm.__exit__(None, None, None)

            # identity + ones columns (first needed by v_finish)
            ident = wts.tile([P, P], BF16, tag="ident")
            from concourse.masks import make_identity

            make_identity(nc, ident)
            nc.any.memset(v_aug[:, :, DV : DV + 1], 1.0)
            nc.any.memset(v_aug[:, :, 2 * DV + 1 : 2 * DV + 2], 1.0)

            # ---- attention with interleaved V / Q-half1 projections ----
            ps_at_cm = tc.tile_pool(name="ps_at", bufs=1, space="PSUM")
            ps_at = ps_at_cm.__enter__()
            ctx_ps = {}

            def scores_tt(ci, tt):
                c0 = sum(CHUNKS[:ci])
                exs = []
                for half in range(CHUNKS[ci] // NQ):
                    m = ps_at.tile([P, 1024], F32, tag="mega", bufs=2, name="m")
                    s0 = c0 + half * NQ
                    for h in range(HPC):
                        nc.tensor.matmul(
                            m[:, h * NQ : (h + 1) * NQ],
                            kt_sb[h * DK : (h + 1) * DK, tt * P : (tt + 1) * P],
                            qt_sb[h * DK : (h + 1) * DK, s0 : s0 + NQ],
                            start=True,
                            stop=True,
                        )
                    ex = expp.tile(
                        [P, 1024], BF16, tag=f"exp{len(exs) % 2}", bufs=16,
                        name="ex",
                    )
                    nc.scalar.activation(
                        ex[:], m[:], EXPF, scale=1.0 / np.sqrt(DK)
                    )
                    exs.append(ex)
                return exs

            def ctx_tt(ci, tt, exs):
                for h in range(HPC):
                    for half, ex in enumerate(exs):
                        nc.tensor.matmul(
                            ctx_ps[(ci, h)][:, half * NQ : (half + 1) * NQ],
                            v_aug[:, tt, h * (DV + 1) : (h + 1) * (DV + 1)],
                            ex[:, h * NQ : (h + 1) * NQ],
                            start=(tt == 0),
                            stop=(tt == TT - 1),
                        )

            def normalize(ci):
                cw = CHUNKS[ci]
                c0 = sum(CHUNKS[:ci])
                for h in range(HPC):
                    den = misc.tile([1, cw], F32, tag="den", name="den")
                    nc.vector.tensor_copy(
                        den[:], ctx_ps[(ci, h)][DV : DV + 1, 0:cw]
                    )
                    recip = misc.tile([1, cw], F32, tag="recip", name="recip")
                    nc.vector.reciprocal_approx_fast(recip[:], den[:])
                    bcast = misc.tile([DV, cw], F32, tag="bcast", name="bcast")
                    nc.gpsimd.partition_broadcast(bcast[:], recip[:])
                    nc.vector.tensor_mul(
                        cat_loc[h * DV : (h + 1) * DV, c0 : c0 + cw],
                        ctx_ps[(ci, h)][0:DV, 0:cw],
                        bcast[:],
                    )
                cb = dram.tile([P, cw], BF16, tag=f"catb{ci}", name="cb")
                nc.sync.dma_start(cb[:], cat_loc[:, c0 : c0 + cw])
                ga = dram.tile([D, cw], BF16, tag=f"catall{ci}", name="ga")
                nc.gpsimd.collective_compute(
                    "AllGather",
                    mybir.AluOpType.bypass,
                    ins=[cb[:].opt()],
                    outs=[ga[:].opt()],
                    replica_groups=rg,
                )
                return ga

            def outproj(ci, ga):
                cw = CHUNKS[ci]
                c0 = sum(CHUNKS[:ci])
                m = ps_at.tile([P, 1024], F32, tag="mega", bufs=2, name="om")
                for kt in range(KT_D):
                    ct = catin.tile([P, cw], BF16, tag="catkt", name="ct")
                    nc.sync.dma_start(ct[:], ga[kt * P : (kt + 1) * P, :])
                    for nn in range(cw // NQ):
                        nc.tensor.matmul(
                            m[:, nn * NQ : (nn + 1) * NQ],
                            wtiles["wo"][:, kt, :],
                            ct[:, nn * NQ : (nn + 1) * NQ],
                            start=(kt == 0),
                            stop=(kt == KT_D - 1),
                        )
                nc.vector.tensor_copy(out_sb[:, c0 : c0 + cw], m[:, 0:cw])
                nc.sync.dma_start(
                    out_t[:, c0 : c0 + cw], out_sb[:, c0 : c0 + cw]
                )

            # --- chunk 0 scores, V then Q-half1 interleaved ---
            ps_v2_cm = tc.tile_pool(name="ps_v2", bufs=1, space="PSUM")
            ps_v2 = ps_v2_cm.__enter__()
            vacc = {
                half: ps_v2.tile(
                    [P, 1024], F32, tag=f"va{half}", name=f"va{half}"
                )
                for half in range(2)
            }

            def v_partial(dt):
                ev = load_enc("v", dt, (0, S), "v", bufs=2)
                for half in range(2):
                    for nn in range(2):
                        off = half * 1024 + nn * NQ
                        nc.tensor.matmul(
                            vacc[half][:, nn * NQ : (nn + 1) * NQ],
                            wtiles["wv"][:, dt, :],
                            ev[:, off : off + NQ],
                            start=(dt == 0),
                            stop=(dt == KT_D - 1),
                        )

            def v_finish():
                for half in range(2):
                    for nn in range(2):
                        off = half * 1024 + nn * NQ
                        nc.vector.tensor_copy(
                            vt_sb[:, off : off + NQ],
                            vacc[half][:, nn * NQ : (nn + 1) * NQ],
                        )
                for tt in range(TT):
                    tp = ps_v2.tile([P, P], BF16, tag="va0", name="tp")
                    nc.tensor.transpose(
                        tp[:], vt_sb[:, tt * P : (tt + 1) * P], ident[:]
                    )
                    nc.vector.tensor_copy(v_aug[:, tt, 0:DV], tp[:, 0:DV])
                    nc.vector.tensor_copy(
                        v_aug[:, tt, DV + 1 : 2 * DV + 1], tp[:, DV : 2 * DV]
                    )

            sc0_exs = []
            ps_q2_cm = None
            for tt in range(TT):
                sc0_exs.append(scores_tt(0, tt))
                if tt < 8:
                    v_partial(tt)
                elif tt == 8:
                    v_finish()
                    ps_v2_cm.__exit__(None, None, None)
                    ps_q2_cm = tc.tile_pool(name="ps_q2", bufs=1, space="PSUM")
                    ps_q2 = ps_q2_cm.__enter__()
                    q2acc = ps_q2.tile([P, 1024], F32, tag="q2a", name="q2a")
                elif tt < 15:
                    dt = tt - 9  # 0..5
                    eq = load_enc("q", dt, (1024, 2048), "q")
                    for nn in range(2):
                        nc.tensor.matmul(
                            q2acc[:, nn * NQ : (nn + 1) * NQ],
                            wtiles["wq"][:, dt, :],
                            eq[:, nn * NQ : (nn + 1) * NQ],
                            start=(dt == 0),
                            stop=False,
                        )
            for dt in (6, 7):
                eq = load_enc("q", dt, (1024, 2048), "q")
                for nn in range(2):
                    nc.tensor.matmul(
                        q2acc[:, nn * NQ : (nn + 1) * NQ],
                        wtiles["wq"][:, dt, :],
                        eq[:, nn * NQ : (nn + 1) * NQ],
                        start=False,
                        stop=(dt == 7),
                    )
            for nn in range(2):
                nc.vector.tensor_copy(
                    qt_sb[:, 1024 + nn * NQ : 1024 + (nn + 1) * NQ],
                    q2acc[:, nn * NQ : (nn + 1) * NQ],
                )
            ps_q2_cm.__exit__(None, None, None)

            # ctx pool (time-shares the banks V/Q-half1 used)
            ps_cx_cm = tc.tile_pool(name="ps_cx", bufs=1, space="PSUM")
            ps_cx = ps_cx_cm.__enter__()

            def alloc_ctx(ci):
                for h in range(HPC):
                    ctx_ps[(ci, h)] = ps_cx.tile(
                        [DV + 1, 1024], F32, tag=f"cx{h}", name=f"cx{h}"
                    )

            # --- chunk 1 scores + ctx(0) at double rate ---
            alloc_ctx(0)
            sc1_exs = []
            for tt in range(TT):
                sc1_exs.append(scores_tt(1, tt))
                for k in (2 * tt, 2 * tt + 1):
                    if k < TT:
                        ctx_tt(0, k, sc0_exs[k])
            ga0 = normalize(0)

            # --- chunk 2 scores + ctx(1) ---
            alloc_ctx(1)
            sc2_exs = []
            for tt in range(TT):
                sc2_exs.append(scores_tt(2, tt))
                ctx_tt(1, tt, sc1_exs[tt])
            ga1 = normalize(1)
            alloc_ctx(2)
            for tt in range(TT):
                ctx_tt(2, tt, sc2_exs[tt])
            ga2 = normalize(2)
            outproj(0, ga0)
            outproj(1, ga1)
            outproj(2, ga2)
            ps_cx_cm.__exit__(None, None, None)
            ps_at_cm.__exit__(None, None, None)

    nc.compile()
    return nc


def kernel(
    encodings_for_q,
    encodings_for_k,
    encodings_for_v,
    W_q,
    W_k,
    W_v,
    W_out,
    _trace: bool = False,
):
    encodings_for_q = np.asarray(encodings_for_q, dtype=np.float32)
    encodings_for_k = np.asarray(encodings_for_k, dtype=np.float32)
    encodings_for_v = np.asarray(encodings_for_v, dtype=np.float32)
    W_q = np.asarray(W_q, dtype=np.float32)
    W_k = np.asarray(W_k, dtype=np.float32)
    W_v = np.asarray(W_v, dtype=np.float32)
    W_out = np.asarray(W_out, dtype=np.float32)

    if "nc" not in _cache:
        _cache["nc"] = build()
    nc = _cache["nc"]

    eqT = np.ascontiguousarray(encodings_for_q.T)
    ekT = np.ascontiguousarray(encodings_for_k.T)
    evT = np.ascontiguousarray(encodings_for_v.T)

    in_maps = []
    for c in range(NCORES):
        hs = slice(HPC * c, HPC * (c + 1))
        in_maps.append(
            {
                "encq_t": eqT,
                "enck_t": ekT,
                "encv_t": evT,
                "wq": _prep_w(np.transpose(W_q[hs], (1, 0, 2)).reshape(D, FW)),
                "wk": _prep_w(np.transpose(W_k[hs], (1, 0, 2)).reshape(D, FW)),
                "wv": _prep_w(np.transpose(W_v[hs], (1, 0, 2)).reshape(D, FW)),
                "wo": _prep_w(W_out[:, FW * c : FW * (c + 1)]),
            }
        )

    r = run_bass_kernel_spmd(
        nc, in_maps, core_ids=list(range(NCORES)), trace=_trace
    )
    out = np.concatenate(
        [r.results[c]["outT"].T for c in range(NCORES)], axis=1
    )
    if _trace:
        kernel.last_exec_time_ns = r.exec_time_ns
        kernel.last_insts = (
            r.instructions_and_trace[0] if r.instructions_and_trace else None
        )
    return out.astype(np.float32)
